# revision 9
# baseline (speedup 1.0000x reference)
"""Trainium2 Bass kernel for a 3x3 stride-1 pad-1 conv:
x (32,128,64,64) f32, weight (256,128,3,3) f32, bias (256,) f32
-> out (32,256,64,64) f32.

Strategy: data-parallel over batch across 8 NeuronCores (4 samples each).
Per core, the conv is 9 shifted matmuls accumulating in PSUM:
  out[co, hw] = sum_{kh,kw} W[co, :, kh, kw] @ xpad[:, h+kh, w+kw]
C_in=128 sits on the SBUF partition dim; the moving operand is a
[128, 8*64] window of the zero-padded image (rows strided by 66), and
the stationary operand is the [ci, co] slice of one (kh,kw) weight tap,
pre-transposed and cast to bf16 on the host; x is also cast to bf16
host-side (halves its DMA) so both matmul operands are bf16 (the PE
rejects mixed 32-bit/16-bit operands; PSUM accumulation stays fp32).

Loop order is weight-stationary: for each (sample, co-block) the kernel
sweeps k=0..8 over all 8 PSUM banks, so consecutive matmuls reuse the
same stationary operand. PSUM tiles are drained by the otherwise-idle
ACT engine (activation Identity with per-partition bias), freeing the
DVE for the x pad/cast copies. The last round runs tile-major so its
drains stagger instead of bunching at the end.
"""

import numpy as np
import ml_dtypes

import concourse.bass as bass
from concourse import bacc
import concourse.mybir as mybir
import concourse.tile as tile
from concourse.bass_utils import run_bass_kernel_spmd

N_CORES = 8
B_FULL = 32
B_LOCAL = B_FULL // N_CORES  # 4
CI = 128
CO = 256
H = W = 64
HP = WP = 66  # zero-padded image
ROWS = 8  # output rows per PSUM tile -> free dim 8*64 = 512 (walrus max)
N_T = H // ROWS  # psum tiles per (sample, cb)
F32 = mybir.dt.float32
F32R = mybir.dt.float32r
BF16 = mybir.dt.bfloat16
ACT_IDENT = mybir.ActivationFunctionType.Identity

# sample-0/cb-0 is processed in part-rounds of PAIR tiles each, chased by
# x chunks, so the first matmuls only wait for a small slice of the DMA.
PAIR = 2


def build_nc():
    nc = bacc.Bacc()
    x_d = nc.dram_tensor("x", [B_LOCAL, CI, H, W], BF16, kind="ExternalInput")
    w_d = nc.dram_tensor("wt", [CI, 18, 128], BF16, kind="ExternalInput")
    b_d = nc.dram_tensor("bias", [CO], F32, kind="ExternalInput")
    o_d = nc.dram_tensor("out", [B_LOCAL, CO, H, W], BF16, kind="ExternalOutput")

    with tile.TileContext(nc) as tc:
        with (
            tc.tile_pool(name="const", bufs=1) as const,
            tc.tile_pool(name="xstage", bufs=B_LOCAL) as xstage,
            tc.tile_pool(name="xpad", bufs=B_LOCAL) as xpool,
            tc.tile_pool(name="obuf", bufs=6) as opool,
            tc.tile_pool(name="psum", bufs=8, space="PSUM") as pspool,
        ):
            # All input loads ride the ACT HWDGE ring; output stores ride
            # the sync ring. Ring order = need order: x0 chunk 0, per-k
            # cb0 weights, bias, x0 chunks 1.., cb1 weights, x1..x3.
            #
            # PE clock ramps to full speed only after sustained activity;
            # burn the initial DMA wait on dummy matmuls over memset tiles
            # (no DMA dependency) so real matmuls start at a higher clock.
            wz = const.tile([128, 128], BF16)
            nc.vector.memset(wz, 0.0)
            for _ in range(24):
                warm = pspool.tile([128, 128], F32, tag="ps")
                nc.tensor.matmul(warm, wz, wz, start=True, stop=True)

            x_v = x_d.rearrange("b c h w -> b c (h w)")
            zrow = const.tile([128, WP], BF16)
            nc.vector.memset(zrow, 0.0)

            def alloc_sample():
                xin = xstage.tile([128, H * W], BF16)
                xp = xpool.tile([128, HP, WP], BF16)
                nc.vector.tensor_copy(xp[:, 0, :], zrow)
                nc.vector.tensor_copy(xp[:, HP - 1, :], zrow)
                nc.vector.tensor_copy(xp[:, :, 0], zrow)
                nc.vector.tensor_copy(xp[:, :, WP - 1], zrow)
                return xin, xp

            def load_chunk(xin, xp, b, r0, rows):
                nc.scalar.dma_start(
                    xin[:, r0 * W : (r0 + rows) * W],
                    x_v[b, :, r0 * W : (r0 + rows) * W],
                )
                nc.vector.tensor_copy(
                    xp[:, 1 + r0 : 1 + r0 + rows, 1 : W + 1],
                    xin[:, r0 * W : (r0 + rows) * W].rearrange(
                        "p (h w) -> p h w", w=W
                    ),
                )

            # sample 0 chunk c covers the input rows part-round c reads:
            # part-round c handles tiles [c*PAIR, (c+1)*PAIR), whose padded
            # rows end at (c+1)*PAIR*ROWS + 2 -> input rows < that.
            xin0, xp0 = alloc_sample()
            s0_parts = [(0, 1), (1, 1), (2, 2), (4, 2), (6, 2)]
            bounds = [0]
            for t0, nt in s0_parts:
                bounds.append(min((t0 + nt) * ROWS + 2, H))
            load_chunk(xin0, xp0, 0, bounds[0], bounds[1] - bounds[0])

            w_t = const.tile([128, 18, 128], BF16)  # [ci, cb*9+k, co_p]
            for k in range(9):
                nc.scalar.dma_start(w_t[:, k : k + 1], w_d[:, k : k + 1])
            bias_sb = const.tile([128, 2], F32)
            nc.scalar.dma_start(bias_sb, b_d.rearrange("(cb cp) -> cp cb", cb=2))

            for c in range(1, len(s0_parts)):
                load_chunk(xin0, xp0, 0, bounds[c], bounds[c + 1] - bounds[c])
            nc.scalar.dma_start(w_t[:, 9:18], w_d[:, 9:18])
            xps = [xp0]
            for b in range(1, B_LOCAL):
                xin, xp = alloc_sample()
                load_chunk(xin, xp, b, 0, H)
                xps.append(xp)

            o_v = o_d.rearrange("b (cb cp) h w -> b cb cp (h w)", cb=2)

            def drain(b, cb, t):
                h0 = t * ROWS
                ob = opool.tile(
                    [128, ROWS * W], BF16, tag="ob", name=f"ob_{b}_{cb}_{t}"
                )
                nc.scalar.activation(
                    ob, drain.ps[t], ACT_IDENT, bias=bias_sb[:, cb : cb + 1]
                )
                nc.sync.dma_start(o_v[b, cb, :, h0 * W : (h0 + ROWS) * W], ob)

            def round_(b, cb, t0, nt, order="k"):
                """Matmul sweep over tiles [t0, t0+nt).

                order='k': weight-stationary (k outer, tiles inner).
                order='t': tile-major (9 matmuls per tile, staggered drains).
                """
                xp = xps[b]
                pss = {
                    t0 + i: pspool.tile(
                        [128, ROWS * W], F32, tag="ps", name=f"ps_{b}_{cb}_{t0+i}"
                    )
                    for i in range(nt)
                }
                drain.ps = pss

                def mm(t, k):
                    kh, kw = divmod(k, 3)
                    h0 = t * ROWS
                    rhs = xp[:, h0 + kh : h0 + kh + ROWS, kw : kw + W]
                    nc.tensor.matmul(
                        pss[t],
                        w_t[:, cb * 9 + k, :],
                        rhs,
                        start=(k == 0),
                        stop=(k == 8),
                    )

                if order == "k":
                    for k in range(9):
                        for i in range(nt):
                            mm(t0 + i, k)
                    for i in range(nt):
                        drain(b, cb, t0 + i)
                else:
                    for i in range(nt):
                        for k in range(9):
                            mm(t0 + i, k)
                        drain(b, cb, t0 + i)

            # sample 0 / cb 0 in small part-rounds chasing the x chunks;
            # middle rounds weight-stationary; last round tile-major so the
            # final drains stagger.
            for t0, nt in s0_parts:
                round_(0, 0, t0, nt)
            round_(0, 1, 0, N_T)
            for b in range(1, B_LOCAL):
                for cb in range(2):
                    last = b == B_LOCAL - 1 and cb == 1
                    round_(b, cb, 0, N_T, order="t" if last else "k")

    nc.finalize()
    return nc


def _prep_x(x: np.ndarray) -> np.ndarray:
    return np.ascontiguousarray(x, dtype=np.float32).astype(ml_dtypes.bfloat16)


def _prep_weight(weight: np.ndarray) -> np.ndarray:
    # [co, ci, kh, kw] -> [ci, cb, kh*kw, co_p] -> [ci, 18, 128] bf16
    w = weight.reshape(2, 128, CI, 9)
    w = w.transpose(2, 0, 3, 1)  # [ci, cb, k, co_p]
    w = np.ascontiguousarray(w.reshape(CI, 18, 128), dtype=np.float32)
    return w.astype(ml_dtypes.bfloat16)


def run(x: np.ndarray, weight: np.ndarray, bias: np.ndarray, **spmd_kwargs):
    x = _prep_x(np.asarray(x))
    wt = _prep_weight(np.asarray(weight))
    bias = np.ascontiguousarray(bias, dtype=np.float32)

    nc = build_nc()
    in_maps = [
        {
            "x": x[c * B_LOCAL : (c + 1) * B_LOCAL],
            "wt": wt,
            "bias": bias,
        }
        for c in range(N_CORES)
    ]
    res = run_bass_kernel_spmd(
        nc, in_maps, core_ids=list(range(N_CORES)), **spmd_kwargs
    )
    out = np.concatenate(
        [np.asarray(r["out"]).astype(np.float32) for r in res.results], axis=0
    )
    return out, res


def kernel(x: np.ndarray, weight: np.ndarray, bias: np.ndarray) -> np.ndarray:
    out, _ = run(x, weight, bias)
    return out


# revision 10
# speedup vs baseline: 1.0022x; 1.0022x over previous
"""Trainium2 Bass kernel for a 3x3 stride-1 pad-1 conv:
x (32,128,64,64) f32, weight (256,128,3,3) f32, bias (256,) f32
-> out (32,256,64,64) f32.

Strategy: data-parallel over batch across 8 NeuronCores (4 samples each).
Per core, the conv is 9 shifted matmuls accumulating in PSUM:
  out[co, hw] = sum_{kh,kw} W[co, :, kh, kw] @ xpad[:, h+kh, w+kw]
C_in=128 sits on the SBUF partition dim; the moving operand is a
[128, 8*64] window of the zero-padded image (rows strided by 66), and
the stationary operand is the [ci, co] slice of one (kh,kw) weight tap,
pre-transposed and cast to bf16 on the host; x is also cast to bf16
host-side (halves its DMA) so both matmul operands are bf16 (the PE
rejects mixed 32-bit/16-bit operands; PSUM accumulation stays fp32).

Loop order is weight-stationary: for each (sample, co-block) the kernel
sweeps k=0..8 over all 8 PSUM banks, so consecutive matmuls reuse the
same stationary operand. PSUM tiles are drained by the otherwise-idle
ACT engine (activation Identity with per-partition bias), freeing the
DVE for the x pad/cast copies. The last round runs tile-major so its
drains stagger instead of bunching at the end.
"""

import numpy as np
import ml_dtypes

import concourse.bass as bass
from concourse import bacc
import concourse.mybir as mybir
import concourse.tile as tile
from concourse.bass_utils import run_bass_kernel_spmd

N_CORES = 8
B_FULL = 32
B_LOCAL = B_FULL // N_CORES  # 4
CI = 128
CO = 256
H = W = 64
HP = WP = 66  # zero-padded image
ROWS = 8  # output rows per PSUM tile -> free dim 8*64 = 512 (walrus max)
N_T = H // ROWS  # psum tiles per (sample, cb)
F32 = mybir.dt.float32
F32R = mybir.dt.float32r
BF16 = mybir.dt.bfloat16
ACT_IDENT = mybir.ActivationFunctionType.Identity

# sample-0/cb-0 is processed in part-rounds of PAIR tiles each, chased by
# x chunks, so the first matmuls only wait for a small slice of the DMA.
PAIR = 2


def build_nc():
    nc = bacc.Bacc()
    x_d = nc.dram_tensor("x", [B_LOCAL, CI, H, W], BF16, kind="ExternalInput")
    w_d = nc.dram_tensor("wt", [CI, 18, 128], BF16, kind="ExternalInput")
    b_d = nc.dram_tensor("bias", [CO], F32, kind="ExternalInput")
    o_d = nc.dram_tensor("out", [B_LOCAL, CO, H, W], BF16, kind="ExternalOutput")

    with tile.TileContext(nc) as tc:
        with (
            tc.tile_pool(name="const", bufs=1) as const,
            tc.tile_pool(name="xstage", bufs=B_LOCAL) as xstage,
            tc.tile_pool(name="xpad", bufs=B_LOCAL) as xpool,
            tc.tile_pool(name="obuf", bufs=6) as opool,
            tc.tile_pool(name="psum", bufs=8, space="PSUM") as pspool,
        ):
            # All input loads ride the ACT HWDGE ring; output stores ride
            # the sync ring. Ring order = need order: x0 chunk 0, per-k
            # cb0 weights, bias, x0 chunks 1.., cb1 weights, x1..x3.
            #
            # PE clock ramps to full speed only after sustained activity;
            # burn the initial DMA wait on dummy matmuls over memset tiles
            # (no DMA dependency) so real matmuls start at a higher clock.
            wz = const.tile([128, 128], BF16)
            nc.vector.memset(wz, 0.0)
            for _ in range(24):
                warm = pspool.tile([128, 128], F32, tag="ps")
                nc.tensor.matmul(warm, wz, wz, start=True, stop=True)

            x_v = x_d.rearrange("b c h w -> b c (h w)")
            zrow = const.tile([128, WP], BF16)
            nc.vector.memset(zrow, 0.0)

            def alloc_sample():
                xin = xstage.tile([128, H * W], BF16)
                xp = xpool.tile([128, HP, WP], BF16)
                nc.vector.tensor_copy(xp[:, 0, :], zrow)
                nc.vector.tensor_copy(xp[:, HP - 1, :], zrow)
                nc.vector.tensor_copy(xp[:, :, 0], zrow)
                nc.vector.tensor_copy(xp[:, :, WP - 1], zrow)
                return xin, xp

            def load_chunk(xin, xp, b, r0, rows):
                nc.scalar.dma_start(
                    xin[:, r0 * W : (r0 + rows) * W],
                    x_v[b, :, r0 * W : (r0 + rows) * W],
                )
                nc.vector.tensor_copy(
                    xp[:, 1 + r0 : 1 + r0 + rows, 1 : W + 1],
                    xin[:, r0 * W : (r0 + rows) * W].rearrange(
                        "p (h w) -> p h w", w=W
                    ),
                )

            # sample 0 chunk c covers the input rows part-round c reads:
            # part-round c handles tiles [c*PAIR, (c+1)*PAIR), whose padded
            # rows end at (c+1)*PAIR*ROWS + 2 -> input rows < that.
            xin0, xp0 = alloc_sample()
            s0_parts = [(0, 2), (2, 2), (4, 2), (6, 2)]
            bounds = [0]
            for t0, nt in s0_parts:
                bounds.append(min((t0 + nt) * ROWS + 2, H))
            load_chunk(xin0, xp0, 0, bounds[0], bounds[1] - bounds[0])

            w_t = const.tile([128, 18, 128], BF16)  # [ci, cb*9+k, co_p]
            for k in range(9):
                nc.scalar.dma_start(w_t[:, k : k + 1], w_d[:, k : k + 1])
            bias_sb = const.tile([128, 2], F32)
            nc.scalar.dma_start(bias_sb, b_d.rearrange("(cb cp) -> cp cb", cb=2))

            for c in range(1, len(s0_parts)):
                load_chunk(xin0, xp0, 0, bounds[c], bounds[c + 1] - bounds[c])
            nc.scalar.dma_start(w_t[:, 9:18], w_d[:, 9:18])
            xps = [xp0]
            for b in range(1, B_LOCAL):
                xin, xp = alloc_sample()
                load_chunk(xin, xp, b, 0, H)
                xps.append(xp)

            o_v = o_d.rearrange("b (cb cp) h w -> b cb cp (h w)", cb=2)

            def drain(b, cb, t):
                h0 = t * ROWS
                ob = opool.tile(
                    [128, ROWS * W], BF16, tag="ob", name=f"ob_{b}_{cb}_{t}"
                )
                nc.scalar.activation(
                    ob, drain.ps[t], ACT_IDENT, bias=bias_sb[:, cb : cb + 1]
                )
                nc.sync.dma_start(o_v[b, cb, :, h0 * W : (h0 + ROWS) * W], ob)

            def round_(b, cb, t0, nt, order="k"):
                """Matmul sweep over tiles [t0, t0+nt).

                order='k': weight-stationary (k outer, tiles inner).
                order='t': tile-major (9 matmuls per tile, staggered drains).
                """
                xp = xps[b]
                pss = {
                    t0 + i: pspool.tile(
                        [128, ROWS * W], F32, tag="ps", name=f"ps_{b}_{cb}_{t0+i}"
                    )
                    for i in range(nt)
                }
                drain.ps = pss

                def mm(t, k):
                    kh, kw = divmod(k, 3)
                    h0 = t * ROWS
                    rhs = xp[:, h0 + kh : h0 + kh + ROWS, kw : kw + W]
                    nc.tensor.matmul(
                        pss[t],
                        w_t[:, cb * 9 + k, :],
                        rhs,
                        start=(k == 0),
                        stop=(k == 8),
                    )

                if order == "k":
                    for k in range(9):
                        for i in range(nt):
                            mm(t0 + i, k)
                    for i in range(nt):
                        drain(b, cb, t0 + i)
                else:
                    for i in range(nt):
                        for k in range(9):
                            mm(t0 + i, k)
                        drain(b, cb, t0 + i)

            # sample 0 / cb 0 in small part-rounds chasing the x chunks;
            # middle rounds weight-stationary; last round tile-major so the
            # final drains stagger.
            for t0, nt in s0_parts:
                round_(0, 0, t0, nt)
            round_(0, 1, 0, N_T)
            for b in range(1, B_LOCAL):
                for cb in range(2):
                    last = b == B_LOCAL - 1 and cb == 1
                    round_(b, cb, 0, N_T, order="t" if last else "k")

    nc.finalize()
    return nc


def _prep_x(x: np.ndarray) -> np.ndarray:
    return np.ascontiguousarray(x, dtype=np.float32).astype(ml_dtypes.bfloat16)


def _prep_weight(weight: np.ndarray) -> np.ndarray:
    # [co, ci, kh, kw] -> [ci, cb, kh*kw, co_p] -> [ci, 18, 128] bf16
    w = weight.reshape(2, 128, CI, 9)
    w = w.transpose(2, 0, 3, 1)  # [ci, cb, k, co_p]
    w = np.ascontiguousarray(w.reshape(CI, 18, 128), dtype=np.float32)
    return w.astype(ml_dtypes.bfloat16)


def run(x: np.ndarray, weight: np.ndarray, bias: np.ndarray, **spmd_kwargs):
    x = _prep_x(np.asarray(x))
    wt = _prep_weight(np.asarray(weight))
    bias = np.ascontiguousarray(bias, dtype=np.float32)

    nc = build_nc()
    in_maps = [
        {
            "x": x[c * B_LOCAL : (c + 1) * B_LOCAL],
            "wt": wt,
            "bias": bias,
        }
        for c in range(N_CORES)
    ]
    res = run_bass_kernel_spmd(
        nc, in_maps, core_ids=list(range(N_CORES)), **spmd_kwargs
    )
    out = np.concatenate(
        [np.asarray(r["out"]).astype(np.float32) for r in res.results], axis=0
    )
    return out, res


def kernel(x: np.ndarray, weight: np.ndarray, bias: np.ndarray) -> np.ndarray:
    out, _ = run(x, weight, bias)
    return out


# revision 18
# speedup vs baseline: 1.0354x; 1.0332x over previous
"""Trainium2 Bass kernel for a 3x3 stride-1 pad-1 conv:
x (32,128,64,64) f32, weight (256,128,3,3) f32, bias (256,) f32
-> out (32,256,64,64) f32.

Strategy: data-parallel over batch across 8 NeuronCores (4 samples each).
Per core, the conv is 9 shifted matmuls accumulating in PSUM:
  out[co, hw] = sum_{kh,kw} W[co, :, kh, kw] @ xpad[:, h+kh, w+kw]
C_in=128 sits on the SBUF partition dim; the moving operand is a
[128, 8*64] window of the zero-padded image (rows strided by 66), and
the stationary operand is the [ci, co] slice of one (kh,kw) weight tap,
pre-transposed and cast to bf16 on the host; x is also cast to bf16
host-side (halves its DMA) so both matmul operands are bf16 (the PE
rejects mixed 32-bit/16-bit operands; PSUM accumulation stays fp32).

Loop order is weight-stationary: for each (sample, co-block) the kernel
sweeps k=0..8 over all 8 PSUM banks, so consecutive matmuls reuse the
same stationary operand. PSUM tiles are drained by the otherwise-idle
ACT engine (activation Identity with per-partition bias), freeing the
DVE for the x pad/cast copies. The last round runs tile-major so its
drains stagger instead of bunching at the end.
"""

import numpy as np
import ml_dtypes

from concourse import bacc
import concourse.mybir as mybir
import concourse.tile as tile
from concourse.bass_utils import run_bass_kernel_spmd

N_CORES = 8
B_FULL = 32
B_LOCAL = B_FULL // N_CORES  # 4
CI = 128
CO = 256
H = W = 64
HP = WP = 66  # zero-padded image
ROWS = 8  # output rows per PSUM tile -> free dim 8*64 = 512 (walrus max)
N_T = H // ROWS  # psum tiles per (sample, cb)
F32 = mybir.dt.float32
BF16 = mybir.dt.bfloat16
ACT_IDENT = mybir.ActivationFunctionType.Identity

# sample-0/cb-0 is processed in part-rounds of PAIR tiles each, chased by
# x chunks, so the first matmuls only wait for a small slice of the DMA.
PAIR = 2


def build_nc():
    nc = bacc.Bacc()
    x_d = nc.dram_tensor("x", [B_LOCAL, CI, H, W], BF16, kind="ExternalInput")
    w_d = nc.dram_tensor("wt", [CI, 18, 128], BF16, kind="ExternalInput")
    b_d = nc.dram_tensor("bias", [CO], F32, kind="ExternalInput")
    o_d = nc.dram_tensor("out", [B_LOCAL, CO, H, W], BF16, kind="ExternalOutput")

    with tile.TileContext(nc) as tc:
        with (
            tc.tile_pool(name="const", bufs=1) as const,
            tc.tile_pool(name="xstage", bufs=B_LOCAL) as xstage,
            tc.tile_pool(name="xpad", bufs=B_LOCAL) as xpool,
            tc.tile_pool(name="obuf", bufs=6) as opool,
            tc.tile_pool(name="psum", bufs=8, space="PSUM") as pspool,
        ):
            # All input loads ride the ACT HWDGE ring; output stores ride
            # the sync ring. Ring order = need order: x0 chunk 0, cb0
            # weights (two DMAs), x0 chunk 1, bias, x0 chunks 2.., cb1
            # weights, x1..x3.
            #
            # PE clock ramps to full speed only after sustained activity;
            # burn the initial DMA wait on dummy matmuls over memset tiles
            # (no DMA dependency) so real matmuls start at a higher clock.
            wz = const.tile([128, 128], BF16)
            nc.vector.memset(wz, 0.0)
            for _ in range(24):
                warm = pspool.tile([128, 128], F32, tag="ps")
                nc.tensor.matmul(warm, wz, wz, start=True, stop=True)

            x_v = x_d.rearrange("b c h w -> b c (h w)")
            # zero row used to clear the 1-px border of each padded image
            zrow = const.tile([128, WP], BF16)
            nc.vector.memset(zrow, 0.0)

            def alloc_sample():
                xin = xstage.tile([128, H * W], BF16)
                xp = xpool.tile([128, HP, WP], BF16)
                nc.vector.tensor_copy(xp[:, 0, :], zrow)
                nc.vector.tensor_copy(xp[:, HP - 1, :], zrow)
                nc.vector.tensor_copy(xp[:, :, 0], zrow)
                nc.vector.tensor_copy(xp[:, :, WP - 1], zrow)
                return xin, xp

            def load_chunk(xin, xp, b, r0, rows):
                nc.scalar.dma_start(
                    xin[:, r0 * W : (r0 + rows) * W],
                    x_v[b, :, r0 * W : (r0 + rows) * W],
                )
                nc.vector.tensor_copy(
                    xp[:, 1 + r0 : 1 + r0 + rows, 1 : W + 1],
                    xin[:, r0 * W : (r0 + rows) * W].rearrange(
                        "p (h w) -> p h w", w=W
                    ),
                )

            # sample 0 chunk c covers the input rows part-round c reads:
            # part-round c handles tiles [c*PAIR, (c+1)*PAIR), whose padded
            # rows end at (c+1)*PAIR*ROWS + 2 -> input rows < that.
            xin0, xp0 = alloc_sample()
            s0_parts = [(0, 2), (2, 2), (4, 2), (6, 2)]
            bounds = [0]
            for t0, nt in s0_parts:
                bounds.append(min((t0 + nt) * ROWS + 2, H))
            load_chunk(xin0, xp0, 0, bounds[0], bounds[1] - bounds[0])

            # two weight DMAs (not nine): every DMA trigger costs ~600ns
            # of sequencer issue time, and the x chunks queued behind them
            # must not be delayed.
            w_t = const.tile([128, 18, 128], BF16)  # [ci, cb*9+k, co_p]
            nc.scalar.dma_start(w_t[:, 0:3], w_d[:, 0:3])
            nc.scalar.dma_start(w_t[:, 3:9], w_d[:, 3:9])
            load_chunk(xin0, xp0, 0, bounds[1], bounds[2] - bounds[1])
            bias_sb = const.tile([128, 2], F32)
            nc.scalar.dma_start(bias_sb, b_d.rearrange("(cb cp) -> cp cb", cb=2))
            for c in range(2, len(s0_parts)):
                load_chunk(xin0, xp0, 0, bounds[c], bounds[c + 1] - bounds[c])
            nc.scalar.dma_start(w_t[:, 9:18], w_d[:, 9:18])
            xps = [xp0]
            for b in range(1, B_LOCAL):
                xin, xp = alloc_sample()
                load_chunk(xin, xp, b, 0, H)
                xps.append(xp)

            o_v = o_d.rearrange("b (cb cp) h w -> b cb cp (h w)", cb=2)

            def drain(b, cb, t, rows=ROWS):
                h0 = t * rows
                ob = opool.tile(
                    [128, rows * W], BF16, tag="ob", name=f"ob_{b}_{cb}_{t}"
                )
                nc.scalar.activation(
                    ob, drain.ps[t], ACT_IDENT, bias=bias_sb[:, cb : cb + 1]
                )
                nc.sync.dma_start(o_v[b, cb, :, h0 * W : (h0 + rows) * W], ob)

            def round_(b, cb, t0, nt, order="k", rows=ROWS):
                """Matmul sweep over tiles [t0, t0+nt) of `rows` out rows.

                order='k': weight-stationary (k outer, tiles inner).
                order='t': tile-major (9 matmuls per tile, staggered drains).
                """
                xp = xps[b]
                pss = {
                    t0 + i: pspool.tile(
                        [128, rows * W], F32, tag="ps", name=f"ps_{b}_{cb}_{t0+i}"
                    )
                    for i in range(nt)
                }
                drain.ps = pss

                def mm(t, k):
                    kh, kw = divmod(k, 3)
                    h0 = t * rows
                    rhs = xp[:, h0 + kh : h0 + kh + rows, kw : kw + W]
                    nc.tensor.matmul(
                        pss[t],
                        w_t[:, cb * 9 + k, :],
                        rhs,
                        start=(k == 0),
                        stop=(k == 8),
                    )

                if order == "k":
                    for k in range(9):
                        for i in range(nt):
                            mm(t0 + i, k)
                    for i in range(nt):
                        drain(b, cb, t0 + i, rows)
                else:
                    for i in range(nt):
                        for k in range(9):
                            mm(t0 + i, k)
                        drain(b, cb, t0 + i, rows)

            # sample 0 / cb 0 in small part-rounds chasing the x chunks;
            # middle rounds weight-stationary; last round tile-major so the
            # final drains stagger.
            for t0, nt in s0_parts:
                round_(0, 0, t0, nt)
            round_(0, 1, 0, N_T)
            for b in range(1, B_LOCAL):
                for cb in range(2):
                    if b == B_LOCAL - 1 and cb == 1:
                        # final round: first 6 big tiles, then 4-row tiles so
                        # the last drain+store covers only 1/16 of the round
                        round_(b, cb, 0, 6, order="t")
                        round_(b, cb, 12, 4, order="t", rows=ROWS // 2)
                    else:
                        round_(b, cb, 0, N_T, order="k")

    nc.finalize()
    return nc


def _prep_x(x: np.ndarray) -> np.ndarray:
    return np.ascontiguousarray(x, dtype=np.float32).astype(ml_dtypes.bfloat16)


def _prep_weight(weight: np.ndarray) -> np.ndarray:
    # [co, ci, kh, kw] -> [ci, cb, kh*kw, co_p] -> [ci, 18, 128] bf16
    w = weight.reshape(2, 128, CI, 9)
    w = w.transpose(2, 0, 3, 1)  # [ci, cb, k, co_p]
    w = np.ascontiguousarray(w.reshape(CI, 18, 128), dtype=np.float32)
    return w.astype(ml_dtypes.bfloat16)


def run(x: np.ndarray, weight: np.ndarray, bias: np.ndarray, **spmd_kwargs):
    x = _prep_x(np.asarray(x))
    wt = _prep_weight(np.asarray(weight))
    bias = np.ascontiguousarray(bias, dtype=np.float32)

    nc = build_nc()
    in_maps = [
        {
            "x": x[c * B_LOCAL : (c + 1) * B_LOCAL],
            "wt": wt,
            "bias": bias,
        }
        for c in range(N_CORES)
    ]
    res = run_bass_kernel_spmd(
        nc, in_maps, core_ids=list(range(N_CORES)), **spmd_kwargs
    )
    out = np.concatenate(
        [np.asarray(r["out"]).astype(np.float32) for r in res.results], axis=0
    )
    return out, res


def kernel(x: np.ndarray, weight: np.ndarray, bias: np.ndarray) -> np.ndarray:
    out, _ = run(x, weight, bias)
    return out


# revision 19
# speedup vs baseline: 1.0382x; 1.0027x over previous
"""Trainium2 Bass kernel for a 3x3 stride-1 pad-1 conv:
x (32,128,64,64) f32, weight (256,128,3,3) f32, bias (256,) f32
-> out (32,256,64,64) f32.

Strategy: data-parallel over batch across 8 NeuronCores (4 samples each).
Per core, the conv is 9 shifted matmuls accumulating in PSUM:
  out[co, hw] = sum_{kh,kw} W[co, :, kh, kw] @ xpad[:, h+kh, w+kw]
C_in=128 sits on the SBUF partition dim; the moving operand is a
[128, 8*64] window of the zero-padded image (rows strided by 66), and
the stationary operand is the [ci, co] slice of one (kh,kw) weight tap,
pre-transposed and cast to bf16 on the host; x is also cast to bf16
host-side (halves its DMA) so both matmul operands are bf16 (the PE
rejects mixed 32-bit/16-bit operands; PSUM accumulation stays fp32).

Loop order is weight-stationary: for each (sample, co-block) the kernel
sweeps k=0..8 over all 8 PSUM banks, so consecutive matmuls reuse the
same stationary operand. PSUM tiles are drained by the otherwise-idle
ACT engine (activation Identity with per-partition bias), freeing the
DVE for the x pad/cast copies. The last round runs tile-major so its
drains stagger instead of bunching at the end.
"""

import numpy as np
import ml_dtypes

from concourse import bacc
import concourse.mybir as mybir
import concourse.tile as tile
from concourse.bass_utils import run_bass_kernel_spmd

N_CORES = 8
B_FULL = 32
B_LOCAL = B_FULL // N_CORES  # 4
CI = 128
CO = 256
H = W = 64
HP = WP = 66  # zero-padded image
ROWS = 8  # output rows per PSUM tile -> free dim 8*64 = 512 (walrus max)
N_T = H // ROWS  # psum tiles per (sample, cb)
F32 = mybir.dt.float32
BF16 = mybir.dt.bfloat16
ACT_IDENT = mybir.ActivationFunctionType.Identity

# sample-0/cb-0 is processed in part-rounds of PAIR tiles each, chased by
# x chunks, so the first matmuls only wait for a small slice of the DMA.
PAIR = 2


def build_nc():
    nc = bacc.Bacc()
    x_d = nc.dram_tensor("x", [B_LOCAL, CI, H, W], BF16, kind="ExternalInput")
    w_d = nc.dram_tensor("wt", [CI, 18, 128], BF16, kind="ExternalInput")
    b_d = nc.dram_tensor("bias", [CO], F32, kind="ExternalInput")
    o_d = nc.dram_tensor("out", [B_LOCAL, CO, H, W], BF16, kind="ExternalOutput")

    with tile.TileContext(nc) as tc:
        with (
            tc.tile_pool(name="const", bufs=1) as const,
            tc.tile_pool(name="xstage", bufs=B_LOCAL) as xstage,
            tc.tile_pool(name="xpad", bufs=B_LOCAL) as xpool,
            tc.tile_pool(name="obuf", bufs=6) as opool,
            tc.tile_pool(name="psum", bufs=8, space="PSUM") as pspool,
        ):
            # All input loads ride the ACT HWDGE ring; output stores ride
            # the sync ring. Ring order = need order: x0 chunk 0, cb0
            # weights (two DMAs), x0 chunk 1, bias, x0 chunks 2.., cb1
            # weights, x1..x3.
            #
            # PE clock ramps to full speed only after sustained activity;
            # burn the initial DMA wait on dummy matmuls over memset tiles
            # (no DMA dependency) so real matmuls start at a higher clock.
            wz = const.tile([128, 128], BF16)
            nc.vector.memset(wz, 0.0)
            for _ in range(16):
                warm = pspool.tile([128, 128], F32, tag="ps")
                nc.tensor.matmul(warm, wz, wz, start=True, stop=True)

            x_v = x_d.rearrange("b c h w -> b c (h w)")
            # zero row used to clear the 1-px border of each padded image
            zrow = const.tile([128, WP], BF16)
            nc.vector.memset(zrow, 0.0)

            def alloc_sample():
                xin = xstage.tile([128, H * W], BF16)
                xp = xpool.tile([128, HP, WP], BF16)
                nc.vector.tensor_copy(xp[:, 0, :], zrow)
                nc.vector.tensor_copy(xp[:, HP - 1, :], zrow)
                nc.vector.tensor_copy(xp[:, :, 0], zrow)
                nc.vector.tensor_copy(xp[:, :, WP - 1], zrow)
                return xin, xp

            def load_chunk(xin, xp, b, r0, rows, ring=None):
                (ring or nc.scalar).dma_start(
                    xin[:, r0 * W : (r0 + rows) * W],
                    x_v[b, :, r0 * W : (r0 + rows) * W],
                )
                nc.vector.tensor_copy(
                    xp[:, 1 + r0 : 1 + r0 + rows, 1 : W + 1],
                    xin[:, r0 * W : (r0 + rows) * W].rearrange(
                        "p (h w) -> p h w", w=W
                    ),
                )

            # sample 0 chunk c covers the input rows part-round c reads:
            # part-round c handles tiles [c*PAIR, (c+1)*PAIR), whose padded
            # rows end at (c+1)*PAIR*ROWS + 2 -> input rows < that.
            xin0, xp0 = alloc_sample()
            s0_parts = [(0, 2), (2, 2), (4, 2), (6, 2)]
            bounds = [0]
            for t0, nt in s0_parts:
                bounds.append(min((t0 + nt) * ROWS + 2, H))
            # sample-0 chunks ride the sync ring: it is idle until the
            # first stores (~13us in), so their triggers issue in parallel
            # with the weight/bias triggers on the scalar ring and their
            # completion semaphores don't queue behind later transfers.
            load_chunk(xin0, xp0, 0, bounds[0], bounds[1] - bounds[0], ring=nc.sync)

            # two weight DMAs (not nine): every DMA trigger costs ~600ns
            # of sequencer issue time, and the x chunks queued behind them
            # must not be delayed.
            w_t = const.tile([128, 18, 128], BF16)  # [ci, cb*9+k, co_p]
            nc.scalar.dma_start(w_t[:, 0:3], w_d[:, 0:3])
            nc.scalar.dma_start(w_t[:, 3:9], w_d[:, 3:9])
            load_chunk(xin0, xp0, 0, bounds[1], bounds[2] - bounds[1], ring=nc.sync)
            bias_sb = const.tile([128, 2], F32)
            nc.scalar.dma_start(bias_sb, b_d.rearrange("(cb cp) -> cp cb", cb=2))
            for c in range(2, len(s0_parts)):
                load_chunk(xin0, xp0, 0, bounds[c], bounds[c + 1] - bounds[c], ring=nc.sync)
            nc.scalar.dma_start(w_t[:, 9:18], w_d[:, 9:18])
            xps = [xp0]
            for b in range(1, B_LOCAL):
                xin, xp = alloc_sample()
                load_chunk(xin, xp, b, 0, H)
                xps.append(xp)

            o_v = o_d.rearrange("b (cb cp) h w -> b cb cp (h w)", cb=2)

            def drain(b, cb, t, rows=ROWS):
                h0 = t * rows
                ob = opool.tile(
                    [128, rows * W], BF16, tag="ob", name=f"ob_{b}_{cb}_{t}"
                )
                nc.scalar.activation(
                    ob, drain.ps[t], ACT_IDENT, bias=bias_sb[:, cb : cb + 1]
                )
                nc.sync.dma_start(o_v[b, cb, :, h0 * W : (h0 + rows) * W], ob)

            def round_(b, cb, t0, nt, order="k", rows=ROWS):
                """Matmul sweep over tiles [t0, t0+nt) of `rows` out rows.

                order='k': weight-stationary (k outer, tiles inner).
                order='t': tile-major (9 matmuls per tile, staggered drains).
                """
                xp = xps[b]
                pss = {
                    t0 + i: pspool.tile(
                        [128, rows * W], F32, tag="ps", name=f"ps_{b}_{cb}_{t0+i}"
                    )
                    for i in range(nt)
                }
                drain.ps = pss

                def mm(t, k):
                    kh, kw = divmod(k, 3)
                    h0 = t * rows
                    rhs = xp[:, h0 + kh : h0 + kh + rows, kw : kw + W]
                    nc.tensor.matmul(
                        pss[t],
                        w_t[:, cb * 9 + k, :],
                        rhs,
                        start=(k == 0),
                        stop=(k == 8),
                    )

                if order == "k":
                    for k in range(9):
                        for i in range(nt):
                            mm(t0 + i, k)
                    for i in range(nt):
                        drain(b, cb, t0 + i, rows)
                else:
                    for i in range(nt):
                        for k in range(9):
                            mm(t0 + i, k)
                        drain(b, cb, t0 + i, rows)

            # sample 0 / cb 0 in small part-rounds chasing the x chunks;
            # middle rounds weight-stationary; last round tile-major so the
            # final drains stagger.
            for t0, nt in s0_parts:
                round_(0, 0, t0, nt)
            round_(0, 1, 0, N_T)
            for b in range(1, B_LOCAL):
                for cb in range(2):
                    if b == B_LOCAL - 1 and cb == 1:
                        # final round: first 6 big tiles, then 4-row tiles so
                        # the last drain+store covers only 1/16 of the round
                        round_(b, cb, 0, 6, order="t")
                        round_(b, cb, 12, 4, order="t", rows=ROWS // 2)
                    else:
                        round_(b, cb, 0, N_T, order="k")

    nc.finalize()
    return nc


def _prep_x(x: np.ndarray) -> np.ndarray:
    return np.ascontiguousarray(x, dtype=np.float32).astype(ml_dtypes.bfloat16)


def _prep_weight(weight: np.ndarray) -> np.ndarray:
    # [co, ci, kh, kw] -> [ci, cb, kh*kw, co_p] -> [ci, 18, 128] bf16
    w = weight.reshape(2, 128, CI, 9)
    w = w.transpose(2, 0, 3, 1)  # [ci, cb, k, co_p]
    w = np.ascontiguousarray(w.reshape(CI, 18, 128), dtype=np.float32)
    return w.astype(ml_dtypes.bfloat16)


def run(x: np.ndarray, weight: np.ndarray, bias: np.ndarray, **spmd_kwargs):
    x = _prep_x(np.asarray(x))
    wt = _prep_weight(np.asarray(weight))
    bias = np.ascontiguousarray(bias, dtype=np.float32)

    nc = build_nc()
    in_maps = [
        {
            "x": x[c * B_LOCAL : (c + 1) * B_LOCAL],
            "wt": wt,
            "bias": bias,
        }
        for c in range(N_CORES)
    ]
    res = run_bass_kernel_spmd(
        nc, in_maps, core_ids=list(range(N_CORES)), **spmd_kwargs
    )
    out = np.concatenate(
        [np.asarray(r["out"]).astype(np.float32) for r in res.results], axis=0
    )
    return out, res


def kernel(x: np.ndarray, weight: np.ndarray, bias: np.ndarray) -> np.ndarray:
    out, _ = run(x, weight, bias)
    return out
